# revision 22
# baseline (speedup 1.0000x reference)
"""Distributed TRN2 Bass kernel for AdaptiveGraphTopology pairwise edge MLP.

reference:
    a = emb @ W1a.T ; b = emb @ W1b.T           (W1a, W1b = W1[:, :H], W1[:, H:])
    hidden = relu(a[:,None,:] + b[None,:,:] + b1)      # [N,N,H]
    scores = hidden . W2[0] + b2                       # [N,N]
    weights = sigmoid(scores), zeroed diag
    mask    = (weights > 0.5) & ~eye

Sharding: rows i split across 8 cores (128 rows each); everything else
replicated. No collectives: each core DMAs out its row block, host
concatenates.

Per-core compute (mode "full" = v4, software-pipelined):
    BT[h, j] = b_j[h]        (all j)    -- f32r matmul on device
    CT[h, i] = a_i[h]+b1[h]  (local i)  -- f32r matmul + bias on device
    loop over local i:
      X_i[h, j] = relu(BT[h, j] + CT[h, i])   (DVE/ACT, fp32r out)
      scores[i, :] += w2 . X_i  via fp32r matmul whose stationary is a
      sliding window over Z[128, 256] (w2 at column 128, zeros elsewhere):
      window [128-i : 256-i] places w2 in PE column i, so row i's scores
      land in PSUM partition i and the 128 iterations accumulate a full
      [128, 1024] score block (zero columns contribute exact zeros).
    The diagonal is pushed to -1e30 by pre-initializing the score PSUM
    with accumulating matmuls (stationary -1e30*I, moving a per-core
    one-hot eye_rows matrix), so the epilogue is just:
    weights = sigmoid(scores+b2) (diag -> 0.0), mask = scores > -b2.

Measured facts (this device) driving the structure:
  - The PE streams moving data at ~2.0 cols/ns regardless of dtype
    (f32r/bf16), stationary width, PSUM bank pattern or MM size: the
    256 x 512-col score matmuls are a hard ~66 us floor; everything
    else must hide behind it.
  - tc.For_i puts an all-engine barrier at each iteration end, so
    cross-rep pipelining is impossible. BT/CT/eye-init are
    rep-invariant, so each body computes them for the NEXT rep during
    its epilogue/DMA tail (PE idle there), and the prologue seeds the
    first rep: the body then starts generating X_0 immediately.
"""
import numpy as np

N = 1024
H = 128
NCORES = 8
ROWS = N // NCORES  # 128 rows per core

_cache = {}


def _split_multiwaits(nc, limit=1):
    """This walrus build accepts only ONE semaphore wait/update per
    instruction; Tile emits several. Split extras onto adjacent NoOps."""
    import bass_rust

    f = nc.m.functions[0]
    engines = nc.engines

    def make_nop(engine_type):
        eng = engines[engine_type]
        inst = eng.nop(nofuse=True).ins
        for b in f.blocks:
            lst = b.instructions
            for k in range(len(lst) - 1, -1, -1):
                if lst[k] is inst:
                    lst.pop(k)
                    return inst
        return inst

    n_split = 0
    for b in f.blocks:
        insts = b.instructions
        i = 0
        while i < len(insts):
            inst = insts[i]
            si = inst.sync_info
            if si is None:
                i += 1
                continue
            waits = list(si.on_wait)
            ups = list(si.on_update)
            if len(waits) <= limit and len(ups) <= 1:
                i += 1
                continue
            pre = []
            post = []
            if len(waits) > limit:
                extra, waits = waits[: len(waits) - limit], waits[len(waits) - limit :]
                for w in extra:
                    nop = make_nop(inst.engine)
                    nop.sync_info = bass_rust.SyncInfo(on_wait=[w], on_update=[])
                    pre.append(nop)
            if len(ups) > 1:
                ups, extra_u = ups[:1], ups[1:]
                for u in extra_u:
                    nop = make_nop(inst.engine)
                    nop.sync_info = bass_rust.SyncInfo(on_wait=[], on_update=[u])
                    post.append(nop)
            inst.sync_info = bass_rust.SyncInfo(on_wait=waits, on_update=ups)
            insts[i:i] = pre
            i += len(pre)
            if post:
                insts[i + 1 : i + 1] = post
            n_split += 1
            i += 1
    return n_split


def _build(reps=1, loop_reps=1, mode="full"):
    import concourse.bass as bass
    import concourse.mybir as mybir
    from concourse.tile import TileContext

    nc = bass.Bass(trn_type="TRN2")
    f32 = mybir.dt.float32
    f32r = mybir.dt.float32r
    u8 = mybir.dt.uint8

    emb_t = nc.dram_tensor("emb_t", [H, N], f32, kind="ExternalInput")
    emb_rows_t = nc.dram_tensor("emb_rows_t", [H, ROWS], f32, kind="ExternalInput")
    w1a_t = nc.dram_tensor("w1a_t", [H, H], f32, kind="ExternalInput")
    w1b_t = nc.dram_tensor("w1b_t", [H, H], f32, kind="ExternalInput")
    b1_col = nc.dram_tensor("b1_col", [H, 1], f32, kind="ExternalInput")
    zbuf = nc.dram_tensor("zbuf", [H, 2 * H], f32, kind="ExternalInput")
    b2_col = nc.dram_tensor("b2_col", [H, 1], f32, kind="ExternalInput")
    negb2_col = nc.dram_tensor("negb2_col", [H, 1], f32, kind="ExternalInput")
    # rowcol[k] = global row index of local row k: used to build the one-hot
    # eye matrix on device (iota + is_equal) that injects -BIG into the
    # diagonal score entries via one accumulating matmul
    rowcol = nc.dram_tensor("rowcol", [ROWS, 1], f32, kind="ExternalInput")
    negbig_eye = nc.dram_tensor("negbig_eye", [H, H], f32, kind="ExternalInput")

    w_out = nc.dram_tensor("w_out", [ROWS, N], f32, kind="ExternalOutput")
    m_out = nc.dram_tensor("m_out", [ROWS, N], u8, kind="ExternalOutput")

    with TileContext(nc) as tc:
        with (
            tc.tile_pool(name="const", bufs=1) as cp,
            tc.tile_pool(name="xp", bufs=14) as xp,
            tc.tile_pool(name="pp", bufs=1, space="PSUM") as pp,
        ):
            emba_s = cp.tile([H, 512], f32, tag="emba")
            nc.sync.dma_start(out=emba_s[:], in_=emb_t[:, 0:512])
            embb_s = cp.tile([H, 512], f32, tag="embb")
            nc.sync.dma_start(out=embb_s[:], in_=emb_t[:, 512:1024])
            embr_s = cp.tile([H, ROWS], f32, tag="embr")
            nc.sync.dma_start(out=embr_s[:], in_=emb_rows_t[:])
            w1a_s = cp.tile([H, H], f32, tag="w1a")
            nc.sync.dma_start(out=w1a_s[:], in_=w1a_t[:])
            w1b_s = cp.tile([H, H], f32, tag="w1b")
            nc.sync.dma_start(out=w1b_s[:], in_=w1b_t[:])
            b1_s = cp.tile([H, 1], f32, tag="b1")
            nc.sync.dma_start(out=b1_s[:], in_=b1_col[:])
            z_s = cp.tile([H, 2 * H], f32, tag="z")
            nc.sync.dma_start(out=z_s[:], in_=zbuf[:])
            b2_s = cp.tile([H, 1], f32, tag="b2")
            nc.sync.dma_start(out=b2_s[:], in_=b2_col[:])
            nb2_s = cp.tile([H, 1], f32, tag="nb2")
            nc.sync.dma_start(out=nb2_s[:], in_=negb2_col[:])
            rc_s = cp.tile([ROWS, 1], f32, tag="rc")
            nc.sync.dma_start(out=rc_s[:], in_=rowcol[:])
            nbe_s = cp.tile([H, H], f32, tag="nbe")
            nc.sync.dma_start(out=nbe_s[:], in_=negbig_eye[:])

            # round f32r constants once
            zr_s = cp.tile([H, 2 * H], f32r, tag="zr")
            nc.vector.tensor_copy(zr_s[:], z_s[:])
            nber_s = cp.tile([H, H], f32r, tag="nber")
            nc.vector.tensor_copy(nber_s[:], nbe_s[:])
            # f32r copies of emb / W1 halves: lets BT/CT run as 1-cycle/col
            # f32r matmuls instead of 4-cycle/col f32 (prologue-only cost)
            embar_s = cp.tile([H, 512], f32r, tag="embar")
            nc.vector.tensor_copy(embar_s[:], emba_s[:])
            embbr_s = cp.tile([H, 512], f32r, tag="embbr")
            nc.vector.tensor_copy(embbr_s[:], embb_s[:])
            embrr_s = cp.tile([H, ROWS], f32r, tag="embrr")
            nc.vector.tensor_copy(embrr_s[:], embr_s[:])
            w1ar_s = cp.tile([H, H], f32r, tag="w1ar")
            nc.vector.tensor_copy(w1ar_s[:], w1a_s[:])
            w1br_s = cp.tile([H, H], f32r, tag="w1br")
            nc.vector.tensor_copy(w1br_s[:], w1b_s[:])

            # build the one-hot eye matrix on device: eyr[k, j] = (j == rowcol[k])
            it_s = cp.tile([ROWS, N], f32, tag="it")
            nc.gpsimd.iota(it_s[:], pattern=[[1, N]], base=0,
                           channel_multiplier=0,
                           allow_small_or_imprecise_dtypes=True)
            eyr_s = cp.tile([ROWS, N], f32r, tag="eyr")
            nc.vector.tensor_scalar(
                out=eyr_s[:],
                in0=it_s[:],
                scalar1=rc_s[:],
                scalar2=None,
                op0=mybir.AluOpType.is_equal,
            )

            # warm the PE HAM (clock gate) with dummy f32 matmuls while the
            # large input DMAs land, so prep + early main-loop matmuls run at
            # 2.4 GHz instead of the cold 1.2 GHz
            warm_ps = pp.tile([H, 128], f32, tag="warmp")
            for _w in range(12):
                nc.tensor.matmul(
                    warm_ps[:], w1a_s[:], w1a_s[:], start=True, stop=True
                )

            # force the sigmoid ACT table set to load during prep, so the
            # epilogue sigmoid doesn't pay a ~2.7us mid-kernel table swap
            # (relu/identity are filler entries in every set); reading
            # warm_ps also keeps the warm matmuls alive through DCE
            warm_s = cp.tile([H, 1], f32, tag="warm")
            nc.scalar.activation(
                warm_s[:], warm_ps[:, 0:1], mybir.ActivationFunctionType.Sigmoid
            )

            if mode in ("full", "v4", "v5"):
                # software-pipelined: BT/CT/eye-init are rep-invariant, so
                # each body computes them for the NEXT rep during the DMA
                # tail; gens read the copies produced by the previous rep
                ct_ps = pp.tile([H, ROWS], f32, tag="ctp")
                ct_s = cp.tile([H, ROWS], f32, tag="ct")
                bt_ps = pp.tile([H, N], f32, tag="btp")
                bt_s = cp.tile([H, N], f32, tag="bt")
                sc_ps = pp.tile([ROWS, N], f32, tag="scores")
                sig_s = cp.tile([ROWS, N], f32, tag="sig")
                m_s = cp.tile([ROWS, N], u8, tag="m")

                def prep():
                    nc.tensor.matmul(ct_ps[:], w1ar_s[:], embrr_s[:],
                                     start=True, stop=True)
                    nc.scalar.activation(
                        ct_s[:], ct_ps[:],
                        mybir.ActivationFunctionType.Identity, bias=b1_s[:])
                    nc.tensor.matmul(bt_ps[:, 0:512], w1br_s[:], embar_s[:],
                                     start=True, stop=True)
                    nc.tensor.matmul(bt_ps[:, 512:1024], w1br_s[:], embbr_s[:],
                                     start=True, stop=True)
                    nc.vector.tensor_copy(bt_s[:, 0:512], bt_ps[:, 0:512])
                    nc.vector.tensor_copy(bt_s[:, 512:1024], bt_ps[:, 512:1024])
                    for h0 in (0, 512):
                        nc.tensor.matmul(
                            sc_ps[:, h0 : h0 + 512], nber_s[:],
                            eyr_s[:, h0 : h0 + 512],
                            start=True, stop=False, skip_group_check=True)

                def body():
                    for i in range(ROWS):
                        x = xp.tile([H, N], f32r, tag="x")
                        if i % 3 == 1:
                            nc.scalar.activation(
                                x[:], bt_s[:],
                                mybir.ActivationFunctionType.Relu,
                                bias=ct_s[:, i : i + 1])
                        else:
                            nc.vector.tensor_scalar(
                                out=x[:], in0=bt_s[:],
                                scalar1=ct_s[:, i : i + 1], scalar2=0.0,
                                op0=mybir.AluOpType.add,
                                op1=mybir.AluOpType.max)
                        lhsT = zr_s[:, H - i : 2 * H - i]
                        nc.tensor.matmul(
                            sc_ps[:, 0:512], lhsT, x[:, 0:512],
                            start=False, stop=(i == ROWS - 1),
                            skip_group_check=True)
                        nc.tensor.matmul(
                            sc_ps[:, 512:1024], lhsT, x[:, 512:1024],
                            start=False, stop=(i == ROWS - 1),
                            skip_group_check=True)
                    if mode != "v5":
                        for h0 in (0, 512):
                            nc.scalar.activation(
                                sig_s[:, h0 : h0 + 512], sc_ps[:, h0 : h0 + 512],
                                mybir.ActivationFunctionType.Sigmoid, bias=b2_s[:])
                            nc.sync.dma_start(out=w_out[:, h0 : h0 + 512],
                                              in_=sig_s[:, h0 : h0 + 512])
                        nc.vector.tensor_scalar(
                            out=m_s[:], in0=sc_ps[:], scalar1=nb2_s[:],
                            scalar2=None, op0=mybir.AluOpType.is_gt)
                        nc.sync.dma_start(out=m_out[:], in_=m_s[:])
                        prep()
                        return
                    # v5: engine-queue-aware ordering of tail + next-rep prep:
                    # DVE [gens, btcopies, is_gt], ACT [gens, sigmoids, ct],
                    # PE [MMs, BT, CT, eye] so next-rep inputs land earliest
                    for h0 in (0, 512):
                        nc.scalar.activation(
                            sig_s[:, h0 : h0 + 512], sc_ps[:, h0 : h0 + 512],
                            mybir.ActivationFunctionType.Sigmoid, bias=b2_s[:])
                        nc.sync.dma_start(out=w_out[:, h0 : h0 + 512],
                                          in_=sig_s[:, h0 : h0 + 512])
                    nc.tensor.matmul(bt_ps[:, 0:512], w1br_s[:], embar_s[:],
                                     start=True, stop=True)
                    nc.tensor.matmul(bt_ps[:, 512:1024], w1br_s[:], embbr_s[:],
                                     start=True, stop=True)
                    nc.vector.tensor_copy(bt_s[:, 0:512], bt_ps[:, 0:512])
                    nc.vector.tensor_copy(bt_s[:, 512:1024], bt_ps[:, 512:1024])
                    nc.tensor.matmul(ct_ps[:], w1ar_s[:], embrr_s[:],
                                     start=True, stop=True)
                    nc.vector.tensor_scalar(
                        out=m_s[:], in0=sc_ps[:], scalar1=nb2_s[:],
                        scalar2=None, op0=mybir.AluOpType.is_gt)
                    nc.sync.dma_start(out=m_out[:], in_=m_s[:])
                    nc.scalar.activation(
                        ct_s[:], ct_ps[:],
                        mybir.ActivationFunctionType.Identity, bias=b1_s[:])
                    for h0 in (0, 512):
                        nc.tensor.matmul(
                            sc_ps[:, h0 : h0 + 512], nber_s[:],
                            eyr_s[:, h0 : h0 + 512],
                            start=True, stop=False, skip_group_check=True)

                prep()
            elif mode == "v3":
                def body():
                    _body_v3(nc, tc, cp, xp, pp, mybir, f32, f32r, u8,
                             embar_s, embbr_s, embrr_s, w1ar_s, w1br_s, b1_s,
                             zr_s, b2_s, nb2_s, eyr_s, nber_s, w_out, m_out)
            elif mode in ("v2psum", "v2sb"):
                def body():
                    _body_v2(nc, tc, cp, xp, pp, mybir, f32, f32r, u8,
                             embar_s, embbr_s, embrr_s, w1ar_s, w1br_s, b1_s,
                             zr_s, b2_s, nb2_s, eyr_s, nber_s, w_out, m_out,
                             act_src_psum=(mode == "v2psum"))
            else:
                def body():
                    _body_once(nc, tc, cp, xp, pp, mybir, f32, f32r, u8,
                               (emba_s, embb_s), embr_s, w1a_s, w1b_s, b1_s,
                               zr_s, b2_s, nb2_s, eyr_s, nber_s, w_out, m_out,
                               mode)

            if loop_reps > 1:
                with tc.For_i(0, loop_reps, 1):
                    body()
            else:
                for _rep in range(reps):
                    body()

    _split_multiwaits(nc)
    return nc


def _body_v2(nc, tc, cp, xp, pp, mybir, f32, f32r, u8,
             embar_s, embbr_s, embrr_s, w1ar_s, w1br_s, b1_s, zr_s, b2_s,
             nb2_s, eyr_s, nber_s, w_out, m_out, act_src_psum=True):
    """PE-rate-bound body: f32r BT/CT, ACT gens read bt straight from PSUM,
    DVE gens read an SBUF copy; pipelined epilogue halves."""
    # BT = W1b @ embT (f32r matmuls: 1 cyc/col instead of 4)
    bt_ps = pp.tile([H, N], f32, tag="btp")
    nc.tensor.matmul(bt_ps[:, 0:512], w1br_s[:], embar_s[:], start=True, stop=True)
    nc.tensor.matmul(bt_ps[:, 512:1024], w1br_s[:], embbr_s[:], start=True, stop=True)
    # CT = W1a @ embT_rows + b1
    ct_ps = pp.tile([H, ROWS], f32, tag="ctp")
    nc.tensor.matmul(ct_ps[:], w1ar_s[:], embrr_s[:], start=True, stop=True)
    ct_s = cp.tile([H, ROWS], f32, tag="ct")
    nc.scalar.activation(
        ct_s[:], ct_ps[:], mybir.ActivationFunctionType.Identity, bias=b1_s[:]
    )
    # SBUF copy of bt for the DVE generators (DVE from PSUM would drop to 1x)
    bt_s = cp.tile([H, N], f32, tag="bt")
    nc.vector.tensor_copy(bt_s[:], bt_ps[:])

    # scores PSUM, diagonal pre-initialized to -BIG
    sc_ps = pp.tile([ROWS, N], f32, tag="scores")
    for h0 in (0, 512):
        nc.tensor.matmul(
            sc_ps[:, h0 : h0 + 512], nber_s[:], eyr_s[:, h0 : h0 + 512],
            start=True, stop=False,
        )

    # main loop: ACT tiles early (ACT is ready before the bt SBUF copy lands),
    # then interleave so both engines stay fed; ACT reads bt from PSUM
    # ((N+172)/1.2 vs (N+352)/1.2 from SBUF)
    act_set = {0, 1}
    act_set.update(i for i in range(2, ROWS) if i % 3 == 2)
    for i in range(ROWS):
        x = xp.tile([H, N], f32r, tag="x")
        if i in act_set:
            nc.scalar.activation(
                x[:], bt_ps[:] if act_src_psum else bt_s[:],
                mybir.ActivationFunctionType.Relu,
                bias=ct_s[:, i : i + 1],
            )
        else:
            nc.vector.tensor_scalar(
                out=x[:], in0=bt_s[:],
                scalar1=ct_s[:, i : i + 1], scalar2=0.0,
                op0=mybir.AluOpType.add, op1=mybir.AluOpType.max,
            )
        lhsT = zr_s[:, H - i : 2 * H - i]
        nc.tensor.matmul(
            sc_ps[:, 0:512], lhsT, x[:, 0:512],
            start=False, stop=(i == ROWS - 1),
        )
        nc.tensor.matmul(
            sc_ps[:, 512:1024], lhsT, x[:, 512:1024],
            start=False, stop=(i == ROWS - 1),
        )

    # epilogue: halves so the first w_out DMA overlaps the second sigmoid
    sig_s = cp.tile([ROWS, N], f32, tag="sig")
    for h0 in (0, 512):
        nc.scalar.activation(
            sig_s[:, h0 : h0 + 512], sc_ps[:, h0 : h0 + 512],
            mybir.ActivationFunctionType.Sigmoid, bias=b2_s[:],
        )
        nc.sync.dma_start(out=w_out[:, h0 : h0 + 512], in_=sig_s[:, h0 : h0 + 512])
    m_s = cp.tile([ROWS, N], u8, tag="m")
    nc.vector.tensor_scalar(
        out=m_s[:], in0=sc_ps[:], scalar1=nb2_s[:], scalar2=None,
        op0=mybir.AluOpType.is_gt,
    )
    nc.sync.dma_start(out=m_out[:], in_=m_s[:])


def _body_v3(nc, tc, cp, xp, pp, mybir, f32, f32r, u8,
             embar_s, embbr_s, embrr_s, w1ar_s, w1br_s, b1_s, zr_s, b2_s,
             nb2_s, eyr_s, nber_s, w_out, m_out):
    """fullold dataflow (SBUF-src gens, split bt copies on DVE+ACT) with:
    CT-first head, f32r BT/CT matmuls, pipelined sigmoid/DMA tail."""
    # CT first so ct_s is ready before the first ACT generation
    ct_ps = pp.tile([H, ROWS], f32, tag="ctp")
    nc.tensor.matmul(ct_ps[:], w1ar_s[:], embrr_s[:], start=True, stop=True)
    ct_s = cp.tile([H, ROWS], f32, tag="ct")
    nc.scalar.activation(
        ct_s[:], ct_ps[:], mybir.ActivationFunctionType.Identity, bias=b1_s[:]
    )
    bt_ps = pp.tile([H, N], f32, tag="btp")
    nc.tensor.matmul(bt_ps[:, 0:512], w1br_s[:], embar_s[:], start=True, stop=True)
    nc.tensor.matmul(bt_ps[:, 512:1024], w1br_s[:], embbr_s[:], start=True, stop=True)
    bt_s = cp.tile([H, N], f32, tag="bt")
    nc.vector.tensor_copy(bt_s[:, 0:512], bt_ps[:, 0:512])
    nc.scalar.copy(bt_s[:, 512:1024], bt_ps[:, 512:1024])

    sc_ps = pp.tile([ROWS, N], f32, tag="scores")
    for h0 in (0, 512):
        nc.tensor.matmul(
            sc_ps[:, h0 : h0 + 512], nber_s[:], eyr_s[:, h0 : h0 + 512],
            start=True, stop=False,
        )

    for i in range(ROWS):
        x = xp.tile([H, N], f32r, tag="x")
        if i % 3 == 1:
            nc.scalar.activation(
                x[:], bt_s[:], mybir.ActivationFunctionType.Relu,
                bias=ct_s[:, i : i + 1],
            )
        else:
            nc.vector.tensor_scalar(
                out=x[:], in0=bt_s[:],
                scalar1=ct_s[:, i : i + 1], scalar2=0.0,
                op0=mybir.AluOpType.add, op1=mybir.AluOpType.max,
            )
        lhsT = zr_s[:, H - i : 2 * H - i]
        nc.tensor.matmul(
            sc_ps[:, 0:512], lhsT, x[:, 0:512],
            start=False, stop=(i == ROWS - 1),
        )
        nc.tensor.matmul(
            sc_ps[:, 512:1024], lhsT, x[:, 512:1024],
            start=False, stop=(i == ROWS - 1),
        )

    sig_s = cp.tile([ROWS, N], f32, tag="sig")
    for h0 in (0, 512):
        nc.scalar.activation(
            sig_s[:, h0 : h0 + 512], sc_ps[:, h0 : h0 + 512],
            mybir.ActivationFunctionType.Sigmoid, bias=b2_s[:],
        )
        nc.sync.dma_start(out=w_out[:, h0 : h0 + 512], in_=sig_s[:, h0 : h0 + 512])
    m_s = cp.tile([ROWS, N], u8, tag="m")
    nc.vector.tensor_scalar(
        out=m_s[:], in0=sc_ps[:], scalar1=nb2_s[:], scalar2=None,
        op0=mybir.AluOpType.is_gt,
    )
    nc.sync.dma_start(out=m_out[:], in_=m_s[:])


def _body_once(nc, tc, cp, xp, pp, mybir, f32, f32r, u8,
               embt_halves, embr_s, w1a_s, w1b_s, b1_s, zr_s, b2_s, nb2_s,
               eyr_s, nber_s, w_out, m_out, mode="full"):
    emba_s, embb_s = embt_halves
    if mode == "empty":
        return
    if True:
        if True:
            # BT = W1b @ embT  (f32, exact): psum half per matmul; each half
            # depends only on its own emb DMA, and the PSUM->SBUF copies run
            # on different engines so they overlap
            bt_ps = pp.tile([H, N], f32, tag="btp")
            nc.tensor.matmul(
                bt_ps[:, 0:512], w1b_s[:], emba_s[:], start=True, stop=True
            )
            nc.tensor.matmul(
                bt_ps[:, 512:1024], w1b_s[:], embb_s[:], start=True, stop=True
            )
            bt_s = cp.tile([H, N], f32, tag="bt")
            nc.vector.tensor_copy(bt_s[:, 0:512], bt_ps[:, 0:512])
            nc.scalar.copy(bt_s[:, 512:1024], bt_ps[:, 512:1024])

            # CT = W1a @ embT_rows + b1  (f32, exact)
            ct_ps = pp.tile([H, ROWS], f32, tag="ctp")
            nc.tensor.matmul(ct_ps[:], w1a_s[:], embr_s[:], start=True, stop=True)
            ct_s = cp.tile([H, ROWS], f32, tag="ct")
            nc.scalar.activation(
                ct_s[:], ct_ps[:], mybir.ActivationFunctionType.Identity, bias=b1_s[:]
            )

            # main loop: accumulate scores into PSUM [128 rows, 1024 cols]
            sc_ps = pp.tile([ROWS, N], f32, tag="scores")
            # initialize each scores bank with -BIG at the diagonal entries
            # (zeros elsewhere): out[k, j] = -BIG*eye[k, j]; keeps the
            # epilogue off the critical tail
            for h0 in (0, 512):
                nc.tensor.matmul(
                    sc_ps[:, h0 : h0 + 512],
                    nber_s[:],
                    eyr_s[:, h0 : h0 + 512],
                    start=True,
                    stop=False,
                )
            xfix = None
            if mode in ("nogen", "nogen_fixw", "nogen_w32", "nogen_1bank",
                        "nogen_fixw32", "nogen_b8", "nogen_256"):
                xfix = cp.tile([H, N], f32r, tag="xfix")
                nc.vector.tensor_copy(xfix[:, 0:256], zr_s[:])
            if mode == "nogen_256":
                # same total moving cols as nogen, but 512 MMs x 256 cols:
                # separates per-MM fixed overhead from cycle-rate
                lhsT = zr_s[:, 0:H]
                for i in range(2 * ROWS):
                    for c0 in (0, 256):
                        nc.tensor.matmul(
                            sc_ps[:, c0 : c0 + 256], lhsT, xfix[:, c0 : c0 + 256],
                            start=False, stop=(i == 2 * ROWS - 1 and c0 == 256),
                        )
                _epilogue(nc, cp, mybir, f32, u8, sc_ps, b2_s, nb2_s, w_out, m_out)
                return
            if mode == "nogen_bf16":
                # 256 MMs x 512 cols with bf16 moving + bf16 stationary:
                # tests whether the ~250ns/MM is f32r-specific or clock/overhead
                bf16 = mybir.dt.bfloat16
                xbf = cp.tile([H, N], bf16, tag="xbf")
                nc.vector.tensor_copy(xbf[:, 0:256], zr_s[:])
                zbf = cp.tile([H, H], bf16, tag="zbf")
                nc.vector.tensor_copy(zbf[:], zr_s[:, 0:H])
                for i in range(2 * ROWS):
                    nc.tensor.matmul(
                        sc_ps[:, 0:512], zbf[:], xbf[:, 0:512],
                        start=False, stop=(i == 2 * ROWS - 1),
                    )
                _epilogue(nc, cp, mybir, f32, u8, sc_ps, b2_s, nb2_s, w_out, m_out)
                return
            if mode == "nogen_1bank":
                # fixed 128-wide stationary, all MMs -> one PSUM bank
                lhsT = zr_s[:, 0:H]
                for i in range(2 * ROWS):
                    nc.tensor.matmul(
                        sc_ps[:, 0:512], lhsT, xfix[:, 0:512],
                        start=False, stop=(i == 2 * ROWS - 1),
                    )
                _epilogue(nc, cp, mybir, f32, u8, sc_ps, b2_s, nb2_s, w_out, m_out)
                return
            if mode == "nogen_fixw32":
                # fixed 32-wide stationary, all MMs -> one PSUM region
                lhsT = zr_s[:, 128:160]
                for i in range(2 * ROWS):
                    nc.tensor.matmul(
                        sc_ps[0:32, 0:512], lhsT, xfix[:, 0:512],
                        start=False, stop=(i == 2 * ROWS - 1),
                    )
                _epilogue(nc, cp, mybir, f32, u8, sc_ps, b2_s, nb2_s, w_out, m_out)
                return
            if mode == "nogen_b8":
                # sliding stationaries, banks switched every 8 rows
                for i0 in range(0, ROWS, 8):
                    for h0 in (0, 512):
                        for i in range(i0, i0 + 8):
                            lhsT = zr_s[:, H - i : 2 * H - i]
                            nc.tensor.matmul(
                                sc_ps[:, h0 : h0 + 512], lhsT, xfix[:, h0 : h0 + 512],
                                start=False,
                                stop=(i == ROWS - 1),
                            )
                _epilogue(nc, cp, mybir, f32, u8, sc_ps, b2_s, nb2_s, w_out, m_out)
                return
            if mode == "nogen_fixw":
                # PE-only, FIXED stationary: isolates LDWEIGHTS cost vs nogen
                lhsT = zr_s[:, 0:H]
                for i in range(ROWS):
                    nc.tensor.matmul(
                        sc_ps[:, 0:512], lhsT, xfix[:, 0:512],
                        start=False, stop=(i == ROWS - 1),
                    )
                    nc.tensor.matmul(
                        sc_ps[:, 512:1024], lhsT, xfix[:, 512:1024],
                        start=False, stop=(i == ROWS - 1),
                    )
                _epilogue(nc, cp, mybir, f32, u8, sc_ps, b2_s, nb2_s, w_out, m_out)
                return
            if mode == "nogen_w32":
                # PE-only, 32-wide sliding stationaries + tile_position groups
                for g in range(4):
                    for k in range(32):
                        lhsT = zr_s[:, H - k : H + 32 - k]
                        for h0 in (0, 512):
                            nc.tensor.matmul(
                                sc_ps[32 * g : 32 * g + 32, h0 : h0 + 512],
                                lhsT,
                                xfix[:, h0 : h0 + 512],
                                start=False,
                                stop=(k == 31),
                                tile_position=(0, 32 * g),
                            )
                _epilogue(nc, cp, mybir, f32, u8, sc_ps, b2_s, nb2_s, w_out, m_out)
                return
            if mode == "full2":
                # col-group tiled reduction: 32-wide stationaries, 4 strips
                for k in range(32):
                    for g in range(4):
                        i = 32 * g + k
                        x = xp.tile([H, N], f32r, tag="x")
                        if (i * 5) % 13 < 5:
                            nc.scalar.activation(
                                x[:],
                                bt_s[:],
                                mybir.ActivationFunctionType.Relu,
                                bias=ct_s[:, i : i + 1],
                            )
                        else:
                            nc.vector.tensor_scalar(
                                out=x[:],
                                in0=bt_s[:],
                                scalar1=ct_s[:, i : i + 1],
                                scalar2=0.0,
                                op0=mybir.AluOpType.add,
                                op1=mybir.AluOpType.max,
                            )
                        lhsT = zr_s[:, H - k : H + 32 - k]
                        for h0 in (0, 512):
                            nc.tensor.matmul(
                                sc_ps[32 * g : 32 * g + 32, h0 : h0 + 512],
                                lhsT,
                                x[:, h0 : h0 + 512],
                                start=(k == 0),
                                stop=(k == 31),
                                tile_position=(0, 32 * g),
                            )
                _epilogue(nc, cp, mybir, f32, u8, sc_ps, b2_s, nb2_s, w_out, m_out)
                return

            for i in range(ROWS):
                if mode != "nogen":
                    x = xp.tile([H, N], f32r, tag="x")
                    if mode == "actgen" or (mode != "dvegen" and i % 3 == 1):
                        # ACT path: relu(in + bias), ~1147ns
                        nc.scalar.activation(
                            x[:],
                            bt_s[:],
                            mybir.ActivationFunctionType.Relu,
                            bias=ct_s[:, i : i + 1],
                        )
                    else:
                        # DVE path: (in + c_i) then max(.,0), ~720ns
                        nc.vector.tensor_scalar(
                            out=x[:],
                            in0=bt_s[:],
                            scalar1=ct_s[:, i : i + 1],
                            scalar2=0.0,
                            op0=mybir.AluOpType.add,
                            op1=mybir.AluOpType.max,
                        )
                else:
                    x = xfix
                if mode == "nomm":
                    continue
                lhsT = zr_s[:, H - i : 2 * H - i]
                nc.tensor.matmul(
                    sc_ps[:, 0:512],
                    lhsT,
                    x[:, 0:512],
                    start=False,
                    stop=(i == ROWS - 1),
                )
                nc.tensor.matmul(
                    sc_ps[:, 512:1024],
                    lhsT,
                    x[:, 512:1024],
                    start=False,
                    stop=(i == ROWS - 1),
                )
            if mode == "nomm":
                return

            _epilogue(nc, cp, mybir, f32, u8, sc_ps, b2_s, nb2_s, w_out, m_out)


def _epilogue(nc, cp, mybir, f32, u8, sc_ps, b2_s, nb2_s, w_out, m_out):
    # diagonal score entries hold -BIG: sigmoid -> 0 weight, is_gt -> 0 mask
    sig_s = cp.tile([ROWS, N], f32, tag="sig")
    nc.scalar.activation(
        sig_s[:], sc_ps[:], mybir.ActivationFunctionType.Sigmoid, bias=b2_s[:]
    )
    nc.sync.dma_start(out=w_out[:], in_=sig_s[:])

    m_s = cp.tile([ROWS, N], u8, tag="m")
    nc.vector.tensor_scalar(
        out=m_s[:],
        in0=sc_ps[:],
        scalar1=nb2_s[:],
        scalar2=None,
        op0=mybir.AluOpType.is_gt,
    )
    nc.sync.dma_start(out=m_out[:], in_=m_s[:])


def _build_in_maps(inputs):
    node_emb = np.asarray(inputs["node_emb"], dtype=np.float32)
    W1 = np.asarray(inputs["W1"], dtype=np.float32)
    b1 = np.asarray(inputs["b1"], dtype=np.float32)
    W2 = np.asarray(inputs["W2"], dtype=np.float32)
    b2 = np.asarray(inputs["b2"], dtype=np.float32)

    emb_t = np.ascontiguousarray(node_emb.T)  # [H, N]
    w1a_t = np.ascontiguousarray(W1[:, :H].T)  # [e, h]
    w1b_t = np.ascontiguousarray(W1[:, H:].T)
    b1_col = np.ascontiguousarray(b1.reshape(H, 1))
    zbuf = np.zeros((H, 2 * H), dtype=np.float32)
    zbuf[:, H] = W2[0]
    b2v = np.float32(b2.reshape(-1)[0])
    b2_col = np.full((H, 1), b2v, dtype=np.float32)
    negb2_col = -b2_col

    negbig_eye = np.zeros((H, H), dtype=np.float32)
    np.fill_diagonal(negbig_eye, np.float32(-1e30))

    in_maps = []
    for c in range(NCORES):
        r0 = c * ROWS
        in_maps.append(
            {
                "emb_t": emb_t,
                "emb_rows_t": np.ascontiguousarray(emb_t[:, r0 : r0 + ROWS]),
                "w1a_t": w1a_t,
                "w1b_t": w1b_t,
                "b1_col": b1_col,
                "zbuf": zbuf,
                "b2_col": b2_col,
                "negb2_col": negb2_col,
                "rowcol": (r0 + np.arange(ROWS, dtype=np.float32)).reshape(ROWS, 1),
                "negbig_eye": negbig_eye,
            }
        )
    return in_maps


def _make_runner(nc):
    """Build a reusable jitted runner (mirrors bass2jax.run_bass_via_pjrt,
    but cached so repeated kernel() calls skip re-tracing/compiling)."""
    import jax
    import concourse.mybir as mybir
    from jax.sharding import Mesh, PartitionSpec

    try:
        from jax.experimental.shard_map import shard_map
    except ImportError:
        from jax.shard_map import shard_map

    from concourse.bass2jax import (
        _bass_exec_p,
        install_neuronx_cc_hook,
        partition_id_tensor,
    )

    install_neuronx_cc_hook()
    partition_name = nc.partition_id_tensor.name if nc.partition_id_tensor else None

    in_names, out_names, out_avals, zero_outs = [], [], [], []
    for alloc in nc.m.functions[0].allocations:
        if not isinstance(alloc, mybir.MemoryLocationSet):
            continue
        name = alloc.memorylocations[0].name
        if alloc.kind == "ExternalInput":
            if name != partition_name:
                in_names.append(name)
        elif alloc.kind == "ExternalOutput":
            out_names.append(name)
            shape = tuple(alloc.tensor_shape)
            dtype = mybir.dt.np(alloc.dtype)
            out_avals.append(jax.core.ShapedArray(shape, dtype))
            zero_outs.append(np.zeros(shape, dtype))
    n_params = len(in_names)
    all_in_names = list(in_names) + list(out_names)
    if partition_name is not None:
        all_in_names.append(partition_name)

    def _body(*args):
        operands = list(args)
        if partition_name is not None:
            operands.append(partition_id_tensor())
        return tuple(
            _bass_exec_p.bind(
                *operands,
                out_avals=tuple(out_avals),
                in_names=tuple(all_in_names),
                out_names=tuple(out_names),
                lowering_input_output_aliases=(),
                sim_require_finite=True,
                sim_require_nnan=True,
                nc=nc,
            )
        )

    devices = jax.devices()[:NCORES]
    mesh = Mesh(np.asarray(devices), ("core",))
    n_outs = len(out_avals)
    # only these inputs differ per core; the rest are replicated and ship
    # to the devices once instead of 8 concatenated copies
    per_core_names = {"emb_rows_t", "rowcol"}
    in_specs = tuple(
        PartitionSpec("core") if n in per_core_names else PartitionSpec(None)
        for n in in_names
    ) + (PartitionSpec("core"),) * n_outs
    out_specs = (PartitionSpec("core"),) * n_outs
    fn = jax.jit(
        shard_map(_body, mesh=mesh, in_specs=in_specs, out_specs=out_specs,
                  check_rep=False),
        keep_unused=True,
    )
    concat_zeros = [
        np.zeros((NCORES * z.shape[0], *z.shape[1:]), z.dtype) for z in zero_outs
    ]
    return fn, in_names, out_names, out_avals, concat_zeros, per_core_names


def _run_cached(in_maps):
    import jax

    if "runner" not in _cache:
        _cache["runner"] = _make_runner(_cache["nc"])
    fn, in_names, out_names, out_avals, concat_zeros, per_core_names = _cache["runner"]
    concat_in = [
        np.concatenate([np.asarray(m[name]) for m in in_maps], axis=0)
        if name in per_core_names
        else np.asarray(in_maps[0][name])
        for name in in_names
    ]
    out_arrs = fn(*concat_in, *concat_zeros)
    jax.block_until_ready(out_arrs)
    res = {}
    for i, name in enumerate(out_names):
        res[name] = np.asarray(out_arrs[i]).reshape(
            NCORES, *out_avals[i].shape
        )
    return res


def kernel(node_emb, W1, b1, W2, b2, temperature=None, **_ignored):
    import time

    if "nc" not in _cache:
        _cache["nc"] = _build()

    in_maps = _build_in_maps(
        {"node_emb": node_emb, "W1": W1, "b1": b1, "W2": W2, "b2": b2}
    )
    # the device occasionally reports NRT_EXEC_UNIT_UNRECOVERABLE if a prior
    # process wedged it; it self-recovers after ~30s, so retry those (and only
    # those) with backoff
    for attempt in range(3):
        try:
            res = _run_cached(in_maps)
            break
        except Exception as e:  # noqa: BLE001
            msg = str(e)
            transient = (
                "UNRECOVERABLE" in msg
                or "unrecoverable" in msg
                or "UNAVAILABLE" in msg
            )
            if attempt == 2 or not transient:
                raise
            time.sleep(30 * (attempt + 1))
    weights = np.concatenate([res["w_out"][c] for c in range(NCORES)], axis=0)
    mask = np.concatenate([res["m_out"][c] for c in range(NCORES)], axis=0).astype(bool)
    return weights, mask



# revision 31
# speedup vs baseline: 1.1782x; 1.1782x over previous
"""Distributed TRN2 Bass kernel for AdaptiveGraphTopology pairwise edge MLP.

reference:
    a = emb @ W1a.T ; b = emb @ W1b.T           (W1a, W1b = W1[:, :H], W1[:, H:])
    hidden = relu(a[:,None,:] + b[None,:,:] + b1)      # [N,N,H]
    scores = hidden . W2[0] + b2                       # [N,N]
    weights = sigmoid(scores), zeroed diag
    mask    = (weights > 0.5) & ~eye

Sharding: rows i split across 8 cores (128 rows each); everything else
replicated. No collectives: each core DMAs out its row block, host
concatenates.

Per-core compute (mode "full" = v4, software-pipelined):
    BT[h, j] = b_j[h]        (all j)    -- f32r matmul on device
    CT[h, i] = a_i[h]+b1[h]  (local i)  -- f32r matmul + bias on device
    loop over local i:
      X_i[h, j] = relu(BT[h, j] + CT[h, i])   (DVE/ACT, fp32r out)
      scores[i, :] += w2 . X_i  via fp32r matmul whose stationary is a
      sliding window over Z[128, 256] (w2 at column 128, zeros elsewhere):
      window [128-i : 256-i] places w2 in PE column i, so row i's scores
      land in PSUM partition i and the 128 iterations accumulate a full
      [128, 1024] score block (zero columns contribute exact zeros).
    The diagonal is pushed to -1e30 by pre-initializing the score PSUM
    with accumulating matmuls (stationary -1e30*I, moving a per-core
    one-hot eye_rows matrix), so the epilogue is just:
    weights = sigmoid(scores+b2) (diag -> 0.0), mask = scores > -b2.

Measured facts (this device) driving the structure:
  - The PE streams moving data at ~2.0 cols/ns regardless of dtype
    (f32r/bf16), stationary width, PSUM bank pattern or MM size: the
    256 x 512-col score matmuls are a hard ~66 us floor; everything
    else must hide behind it.
  - tc.For_i puts an all-engine barrier at each iteration end, so
    cross-rep pipelining is impossible. BT/CT/eye-init are
    rep-invariant, so each body computes them for the NEXT rep during
    its epilogue/DMA tail (PE idle there), and the prologue seeds the
    first rep: the body then starts generating X_0 immediately.
"""
import numpy as np

N = 1024
H = 128
NCORES = 8
ROWS = N // NCORES  # 128 rows per core

_cache = {}


def _split_multiwaits(nc, limit=1):
    """This walrus build accepts only ONE semaphore wait/update per
    instruction; Tile emits several. Split extras onto adjacent NoOps."""
    import bass_rust

    f = nc.m.functions[0]
    engines = nc.engines

    def make_nop(engine_type):
        eng = engines[engine_type]
        inst = eng.nop(nofuse=True).ins
        for b in f.blocks:
            lst = b.instructions
            for k in range(len(lst) - 1, -1, -1):
                if lst[k] is inst:
                    lst.pop(k)
                    return inst
        return inst

    n_split = 0
    for b in f.blocks:
        insts = b.instructions
        i = 0
        while i < len(insts):
            inst = insts[i]
            si = inst.sync_info
            if si is None:
                i += 1
                continue
            waits = list(si.on_wait)
            ups = list(si.on_update)
            same_sem = (
                len(waits) >= 1 and len(ups) >= 1
                and any(getattr(w, "id", None) == getattr(u, "id", None)
                        for w in waits for u in ups)
            )
            if len(waits) <= limit and len(ups) <= 1 and not same_sem:
                i += 1
                continue
            pre = []
            post = []
            if len(waits) > limit:
                extra, waits = waits[: len(waits) - limit], waits[len(waits) - limit :]
                for w in extra:
                    nop = make_nop(inst.engine)
                    nop.sync_info = bass_rust.SyncInfo(on_wait=[w], on_update=[])
                    pre.append(nop)
            if len(ups) > 1:
                ups, extra_u = ups[:1], ups[1:]
                for u in extra_u:
                    nop = make_nop(inst.engine)
                    nop.sync_info = bass_rust.SyncInfo(on_wait=[], on_update=[u])
                    post.append(nop)
            if (waits and ups
                    and getattr(waits[0], "id", None) == getattr(ups[0], "id", None)):
                # wait+update on one semaphore in a single instruction trips
                # walrus's no_semaphore_value_conflict: hoist the wait onto
                # a preceding NoOp (engine queues are in-order)
                nop = make_nop(inst.engine)
                nop.sync_info = bass_rust.SyncInfo(on_wait=waits, on_update=[])
                pre.append(nop)
                waits = []
            inst.sync_info = bass_rust.SyncInfo(on_wait=waits, on_update=ups)
            insts[i:i] = pre
            i += len(pre)
            if post:
                insts[i + 1 : i + 1] = post
            n_split += 1
            i += 1
    return n_split


def _build(reps=1, loop_reps=1, mode="full"):
    import concourse.bass as bass
    import concourse.mybir as mybir
    from concourse.tile import TileContext

    nc = bass.Bass(trn_type="TRN2")
    f32 = mybir.dt.float32
    f32r = mybir.dt.float32r
    u8 = mybir.dt.uint8

    emb_t = nc.dram_tensor("emb_t", [H, N], f32, kind="ExternalInput")
    emb_rows_t = nc.dram_tensor("emb_rows_t", [H, ROWS], f32, kind="ExternalInput")
    w1a_t = nc.dram_tensor("w1a_t", [H, H], f32, kind="ExternalInput")
    w1b_t = nc.dram_tensor("w1b_t", [H, H], f32, kind="ExternalInput")
    b1_col = nc.dram_tensor("b1_col", [H, 1], f32, kind="ExternalInput")
    zbuf = nc.dram_tensor("zbuf", [H, 2 * H], f32, kind="ExternalInput")
    b2_col = nc.dram_tensor("b2_col", [H, 1], f32, kind="ExternalInput")
    negb2_col = nc.dram_tensor("negb2_col", [H, 1], f32, kind="ExternalInput")
    # rowcol[k] = global row index of local row k: used to build the one-hot
    # eye matrix on device (iota + is_equal) that injects -BIG into the
    # diagonal score entries via one accumulating matmul
    rowcol = nc.dram_tensor("rowcol", [ROWS, 1], f32, kind="ExternalInput")
    negbig_eye = nc.dram_tensor("negbig_eye", [H, H], f32, kind="ExternalInput")

    bf16 = mybir.dt.bfloat16
    # weights leave the core as bf16 (halves the tail DMA); host upcasts.
    # Adds ~1e-3 rel err on weights vs the 2e-2 gate.
    w_out = nc.dram_tensor("w_out", [ROWS, N], bf16, kind="ExternalOutput")
    m_out = nc.dram_tensor("m_out", [ROWS, N], u8, kind="ExternalOutput")

    with TileContext(nc) as tc:
        with (
            tc.tile_pool(name="const", bufs=1) as cp,
            tc.tile_pool(name="xp", bufs=14) as xp,
            tc.tile_pool(name="pp", bufs=1, space="PSUM") as pp,
        ):
            emba_s = cp.tile([H, 512], f32, tag="emba")
            nc.sync.dma_start(out=emba_s[:], in_=emb_t[:, 0:512])
            embb_s = cp.tile([H, 512], f32, tag="embb")
            nc.sync.dma_start(out=embb_s[:], in_=emb_t[:, 512:1024])
            embr_s = cp.tile([H, ROWS], f32, tag="embr")
            nc.sync.dma_start(out=embr_s[:], in_=emb_rows_t[:])
            w1a_s = cp.tile([H, H], f32, tag="w1a")
            nc.sync.dma_start(out=w1a_s[:], in_=w1a_t[:])
            w1b_s = cp.tile([H, H], f32, tag="w1b")
            nc.sync.dma_start(out=w1b_s[:], in_=w1b_t[:])
            b1_s = cp.tile([H, 1], f32, tag="b1")
            nc.sync.dma_start(out=b1_s[:], in_=b1_col[:])
            z_s = cp.tile([H, 2 * H], f32, tag="z")
            nc.sync.dma_start(out=z_s[:], in_=zbuf[:])
            b2_s = cp.tile([H, 1], f32, tag="b2")
            nc.sync.dma_start(out=b2_s[:], in_=b2_col[:])
            nb2_s = cp.tile([H, 1], f32, tag="nb2")
            nc.sync.dma_start(out=nb2_s[:], in_=negb2_col[:])
            rc_s = cp.tile([ROWS, 1], f32, tag="rc")
            nc.sync.dma_start(out=rc_s[:], in_=rowcol[:])
            nbe_s = cp.tile([H, H], f32, tag="nbe")
            nc.sync.dma_start(out=nbe_s[:], in_=negbig_eye[:])

            # round f32r constants once
            zr_s = cp.tile([H, 2 * H], f32r, tag="zr")
            nc.vector.tensor_copy(zr_s[:], z_s[:])
            nber_s = cp.tile([H, H], f32r, tag="nber")
            nc.vector.tensor_copy(nber_s[:], nbe_s[:])
            # f32r copies of emb / W1 halves: lets BT/CT run as 1-cycle/col
            # f32r matmuls instead of 4-cycle/col f32 (prologue-only cost)
            embar_s = cp.tile([H, 512], f32r, tag="embar")
            nc.vector.tensor_copy(embar_s[:], emba_s[:])
            embbr_s = cp.tile([H, 512], f32r, tag="embbr")
            nc.vector.tensor_copy(embbr_s[:], embb_s[:])
            embrr_s = cp.tile([H, ROWS], f32r, tag="embrr")
            nc.vector.tensor_copy(embrr_s[:], embr_s[:])
            w1ar_s = cp.tile([H, H], f32r, tag="w1ar")
            nc.vector.tensor_copy(w1ar_s[:], w1a_s[:])
            w1br_s = cp.tile([H, H], f32r, tag="w1br")
            nc.vector.tensor_copy(w1br_s[:], w1b_s[:])

            # build the one-hot eye matrix on device: eyr[k, j] = (j == rowcol[k])
            it_s = cp.tile([ROWS, N], f32, tag="it")
            nc.gpsimd.iota(it_s[:], pattern=[[1, N]], base=0,
                           channel_multiplier=0,
                           allow_small_or_imprecise_dtypes=True)
            eyr_s = cp.tile([ROWS, N], f32r, tag="eyr")
            nc.vector.tensor_scalar(
                out=eyr_s[:],
                in0=it_s[:],
                scalar1=rc_s[:],
                scalar2=None,
                op0=mybir.AluOpType.is_equal,
            )

            # warm the PE HAM (clock gate) with dummy f32 matmuls while the
            # large input DMAs land, so prep + early main-loop matmuls run at
            # 2.4 GHz instead of the cold 1.2 GHz
            warm_ps = pp.tile([H, 128], f32, tag="warmp")
            for _w in range(12):
                nc.tensor.matmul(
                    warm_ps[:], w1a_s[:], w1a_s[:], start=True, stop=True
                )

            # force the sigmoid ACT table set to load during prep, so the
            # epilogue sigmoid doesn't pay a ~2.7us mid-kernel table swap
            # (relu/identity are filler entries in every set); reading
            # warm_ps also keeps the warm matmuls alive through DCE
            warm_s = cp.tile([H, 1], f32, tag="warm")
            nc.scalar.activation(
                warm_s[:], warm_ps[:, 0:1], mybir.ActivationFunctionType.Sigmoid
            )

            if mode in ("full", "v4", "v4s", "v5"):
                # software-pipelined: BT/CT/eye-init are rep-invariant, so
                # each body computes them for the NEXT rep during the DMA
                # tail; gens read the copies produced by the previous rep
                ct_ps = pp.tile([H, ROWS], f32, tag="ctp")
                ct_s = cp.tile([H, ROWS], f32, tag="ct")
                bt_ps = pp.tile([H, N], f32, tag="btp")
                bt_s = cp.tile([H, N], f32, tag="bt")
                sc_ps = pp.tile([ROWS, N], f32, tag="scores")
                sig_s = cp.tile([ROWS, N], mybir.dt.bfloat16, tag="sig")
                m_s = cp.tile([ROWS, N], u8, tag="m")

                def prep():
                    nc.tensor.matmul(ct_ps[:], w1ar_s[:], embrr_s[:],
                                     start=True, stop=True)
                    nc.scalar.activation(
                        ct_s[:], ct_ps[:],
                        mybir.ActivationFunctionType.Identity, bias=b1_s[:])
                    nc.tensor.matmul(bt_ps[:, 0:512], w1br_s[:], embar_s[:],
                                     start=True, stop=True)
                    nc.tensor.matmul(bt_ps[:, 512:1024], w1br_s[:], embbr_s[:],
                                     start=True, stop=True)
                    nc.vector.tensor_copy(bt_s[:, 0:512], bt_ps[:, 0:512])
                    nc.vector.tensor_copy(bt_s[:, 512:1024], bt_ps[:, 512:1024])
                    for h0 in (0, 512):
                        nc.tensor.matmul(
                            sc_ps[:, h0 : h0 + 512], nber_s[:],
                            eyr_s[:, h0 : h0 + 512],
                            start=True, stop=False, skip_group_check=True)

                def body():
                    for i in range(ROWS):
                        x = xp.tile([H, N], f32r, tag="x")
                        if i % 3 == 1:
                            nc.scalar.activation(
                                x[:], bt_s[:],
                                mybir.ActivationFunctionType.Relu,
                                bias=ct_s[:, i : i + 1])
                        else:
                            nc.vector.tensor_scalar(
                                out=x[:], in0=bt_s[:],
                                scalar1=ct_s[:, i : i + 1], scalar2=0.0,
                                op0=mybir.AluOpType.add,
                                op1=mybir.AluOpType.max)
                        lhsT = zr_s[:, H - i : 2 * H - i]
                        nc.tensor.matmul(
                            sc_ps[:, 0:512], lhsT, x[:, 0:512],
                            start=False, stop=(i == ROWS - 1),
                            skip_group_check=True)
                        nc.tensor.matmul(
                            sc_ps[:, 512:1024], lhsT, x[:, 512:1024],
                            start=False, stop=(i == ROWS - 1),
                            skip_group_check=True)
                    if mode != "v5":
                        for h0 in (0, 512):
                            nc.scalar.activation(
                                sig_s[:, h0 : h0 + 512], sc_ps[:, h0 : h0 + 512],
                                mybir.ActivationFunctionType.Sigmoid, bias=b2_s[:])
                            nc.sync.dma_start(out=w_out[:, h0 : h0 + 512],
                                              in_=sig_s[:, h0 : h0 + 512])
                        nc.vector.tensor_scalar(
                            out=m_s[:], in0=sc_ps[:], scalar1=nb2_s[:],
                            scalar2=None, op0=mybir.AluOpType.is_gt)
                        nc.sync.dma_start(out=m_out[:], in_=m_s[:])
                        prep()
                        return
                    # v5: engine-queue-aware ordering of tail + next-rep prep:
                    # DVE [gens, btcopies, is_gt], ACT [gens, sigmoids, ct],
                    # PE [MMs, BT, CT, eye] so next-rep inputs land earliest
                    for h0 in (0, 512):
                        nc.scalar.activation(
                            sig_s[:, h0 : h0 + 512], sc_ps[:, h0 : h0 + 512],
                            mybir.ActivationFunctionType.Sigmoid, bias=b2_s[:])
                        nc.sync.dma_start(out=w_out[:, h0 : h0 + 512],
                                          in_=sig_s[:, h0 : h0 + 512])
                    nc.tensor.matmul(bt_ps[:, 0:512], w1br_s[:], embar_s[:],
                                     start=True, stop=True)
                    nc.tensor.matmul(bt_ps[:, 512:1024], w1br_s[:], embbr_s[:],
                                     start=True, stop=True)
                    nc.vector.tensor_copy(bt_s[:, 0:512], bt_ps[:, 0:512])
                    nc.vector.tensor_copy(bt_s[:, 512:1024], bt_ps[:, 512:1024])
                    nc.tensor.matmul(ct_ps[:], w1ar_s[:], embrr_s[:],
                                     start=True, stop=True)
                    nc.vector.tensor_scalar(
                        out=m_s[:], in0=sc_ps[:], scalar1=nb2_s[:],
                        scalar2=None, op0=mybir.AluOpType.is_gt)
                    nc.sync.dma_start(out=m_out[:], in_=m_s[:])
                    nc.scalar.activation(
                        ct_s[:], ct_ps[:],
                        mybir.ActivationFunctionType.Identity, bias=b1_s[:])
                    for h0 in (0, 512):
                        nc.tensor.matmul(
                            sc_ps[:, h0 : h0 + 512], nber_s[:],
                            eyr_s[:, h0 : h0 + 512],
                            start=True, stop=False, skip_group_check=True)

                prep()
            elif mode == "v3":
                def body():
                    _body_v3(nc, tc, cp, xp, pp, mybir, f32, f32r, u8,
                             embar_s, embbr_s, embrr_s, w1ar_s, w1br_s, b1_s,
                             zr_s, b2_s, nb2_s, eyr_s, nber_s, w_out, m_out)
            elif mode in ("v2psum", "v2sb"):
                def body():
                    _body_v2(nc, tc, cp, xp, pp, mybir, f32, f32r, u8,
                             embar_s, embbr_s, embrr_s, w1ar_s, w1br_s, b1_s,
                             zr_s, b2_s, nb2_s, eyr_s, nber_s, w_out, m_out,
                             act_src_psum=(mode == "v2psum"))
            else:
                def body():
                    _body_once(nc, tc, cp, xp, pp, mybir, f32, f32r, u8,
                               (emba_s, embb_s), embr_s, w1a_s, w1b_s, b1_s,
                               zr_s, b2_s, nb2_s, eyr_s, nber_s, w_out, m_out,
                               mode)

            if loop_reps > 1:
                # staggered_reset drops the per-iteration all-engine barrier
                # (rolling per-stage sem resets instead), letting engines flow
                # into the next rep while others drain the tail
                with tc.For_i(0, loop_reps, 1,
                              staggered_reset=(mode == "v4s")):
                    body()
            else:
                for _rep in range(reps):
                    body()

    _split_multiwaits(nc)
    return nc


def _body_v2(nc, tc, cp, xp, pp, mybir, f32, f32r, u8,
             embar_s, embbr_s, embrr_s, w1ar_s, w1br_s, b1_s, zr_s, b2_s,
             nb2_s, eyr_s, nber_s, w_out, m_out, act_src_psum=True):
    """PE-rate-bound body: f32r BT/CT, ACT gens read bt straight from PSUM,
    DVE gens read an SBUF copy; pipelined epilogue halves."""
    # BT = W1b @ embT (f32r matmuls: 1 cyc/col instead of 4)
    bt_ps = pp.tile([H, N], f32, tag="btp")
    nc.tensor.matmul(bt_ps[:, 0:512], w1br_s[:], embar_s[:], start=True, stop=True)
    nc.tensor.matmul(bt_ps[:, 512:1024], w1br_s[:], embbr_s[:], start=True, stop=True)
    # CT = W1a @ embT_rows + b1
    ct_ps = pp.tile([H, ROWS], f32, tag="ctp")
    nc.tensor.matmul(ct_ps[:], w1ar_s[:], embrr_s[:], start=True, stop=True)
    ct_s = cp.tile([H, ROWS], f32, tag="ct")
    nc.scalar.activation(
        ct_s[:], ct_ps[:], mybir.ActivationFunctionType.Identity, bias=b1_s[:]
    )
    # SBUF copy of bt for the DVE generators (DVE from PSUM would drop to 1x)
    bt_s = cp.tile([H, N], f32, tag="bt")
    nc.vector.tensor_copy(bt_s[:], bt_ps[:])

    # scores PSUM, diagonal pre-initialized to -BIG
    sc_ps = pp.tile([ROWS, N], f32, tag="scores")
    for h0 in (0, 512):
        nc.tensor.matmul(
            sc_ps[:, h0 : h0 + 512], nber_s[:], eyr_s[:, h0 : h0 + 512],
            start=True, stop=False,
        )

    # main loop: ACT tiles early (ACT is ready before the bt SBUF copy lands),
    # then interleave so both engines stay fed; ACT reads bt from PSUM
    # ((N+172)/1.2 vs (N+352)/1.2 from SBUF)
    act_set = {0, 1}
    act_set.update(i for i in range(2, ROWS) if i % 3 == 2)
    for i in range(ROWS):
        x = xp.tile([H, N], f32r, tag="x")
        if i in act_set:
            nc.scalar.activation(
                x[:], bt_ps[:] if act_src_psum else bt_s[:],
                mybir.ActivationFunctionType.Relu,
                bias=ct_s[:, i : i + 1],
            )
        else:
            nc.vector.tensor_scalar(
                out=x[:], in0=bt_s[:],
                scalar1=ct_s[:, i : i + 1], scalar2=0.0,
                op0=mybir.AluOpType.add, op1=mybir.AluOpType.max,
            )
        lhsT = zr_s[:, H - i : 2 * H - i]
        nc.tensor.matmul(
            sc_ps[:, 0:512], lhsT, x[:, 0:512],
            start=False, stop=(i == ROWS - 1),
        )
        nc.tensor.matmul(
            sc_ps[:, 512:1024], lhsT, x[:, 512:1024],
            start=False, stop=(i == ROWS - 1),
        )

    # epilogue: halves so the first w_out DMA overlaps the second sigmoid
    sig_s = cp.tile([ROWS, N], mybir.dt.bfloat16, tag="sig")
    for h0 in (0, 512):
        nc.scalar.activation(
            sig_s[:, h0 : h0 + 512], sc_ps[:, h0 : h0 + 512],
            mybir.ActivationFunctionType.Sigmoid, bias=b2_s[:],
        )
        nc.sync.dma_start(out=w_out[:, h0 : h0 + 512], in_=sig_s[:, h0 : h0 + 512])
    m_s = cp.tile([ROWS, N], u8, tag="m")
    nc.vector.tensor_scalar(
        out=m_s[:], in0=sc_ps[:], scalar1=nb2_s[:], scalar2=None,
        op0=mybir.AluOpType.is_gt,
    )
    nc.sync.dma_start(out=m_out[:], in_=m_s[:])


def _body_v3(nc, tc, cp, xp, pp, mybir, f32, f32r, u8,
             embar_s, embbr_s, embrr_s, w1ar_s, w1br_s, b1_s, zr_s, b2_s,
             nb2_s, eyr_s, nber_s, w_out, m_out):
    """fullold dataflow (SBUF-src gens, split bt copies on DVE+ACT) with:
    CT-first head, f32r BT/CT matmuls, pipelined sigmoid/DMA tail."""
    # CT first so ct_s is ready before the first ACT generation
    ct_ps = pp.tile([H, ROWS], f32, tag="ctp")
    nc.tensor.matmul(ct_ps[:], w1ar_s[:], embrr_s[:], start=True, stop=True)
    ct_s = cp.tile([H, ROWS], f32, tag="ct")
    nc.scalar.activation(
        ct_s[:], ct_ps[:], mybir.ActivationFunctionType.Identity, bias=b1_s[:]
    )
    bt_ps = pp.tile([H, N], f32, tag="btp")
    nc.tensor.matmul(bt_ps[:, 0:512], w1br_s[:], embar_s[:], start=True, stop=True)
    nc.tensor.matmul(bt_ps[:, 512:1024], w1br_s[:], embbr_s[:], start=True, stop=True)
    bt_s = cp.tile([H, N], f32, tag="bt")
    nc.vector.tensor_copy(bt_s[:, 0:512], bt_ps[:, 0:512])
    nc.scalar.copy(bt_s[:, 512:1024], bt_ps[:, 512:1024])

    sc_ps = pp.tile([ROWS, N], f32, tag="scores")
    for h0 in (0, 512):
        nc.tensor.matmul(
            sc_ps[:, h0 : h0 + 512], nber_s[:], eyr_s[:, h0 : h0 + 512],
            start=True, stop=False,
        )

    for i in range(ROWS):
        x = xp.tile([H, N], f32r, tag="x")
        if i % 3 == 1:
            nc.scalar.activation(
                x[:], bt_s[:], mybir.ActivationFunctionType.Relu,
                bias=ct_s[:, i : i + 1],
            )
        else:
            nc.vector.tensor_scalar(
                out=x[:], in0=bt_s[:],
                scalar1=ct_s[:, i : i + 1], scalar2=0.0,
                op0=mybir.AluOpType.add, op1=mybir.AluOpType.max,
            )
        lhsT = zr_s[:, H - i : 2 * H - i]
        nc.tensor.matmul(
            sc_ps[:, 0:512], lhsT, x[:, 0:512],
            start=False, stop=(i == ROWS - 1),
        )
        nc.tensor.matmul(
            sc_ps[:, 512:1024], lhsT, x[:, 512:1024],
            start=False, stop=(i == ROWS - 1),
        )

    sig_s = cp.tile([ROWS, N], mybir.dt.bfloat16, tag="sig")
    for h0 in (0, 512):
        nc.scalar.activation(
            sig_s[:, h0 : h0 + 512], sc_ps[:, h0 : h0 + 512],
            mybir.ActivationFunctionType.Sigmoid, bias=b2_s[:],
        )
        nc.sync.dma_start(out=w_out[:, h0 : h0 + 512], in_=sig_s[:, h0 : h0 + 512])
    m_s = cp.tile([ROWS, N], u8, tag="m")
    nc.vector.tensor_scalar(
        out=m_s[:], in0=sc_ps[:], scalar1=nb2_s[:], scalar2=None,
        op0=mybir.AluOpType.is_gt,
    )
    nc.sync.dma_start(out=m_out[:], in_=m_s[:])


def _body_once(nc, tc, cp, xp, pp, mybir, f32, f32r, u8,
               embt_halves, embr_s, w1a_s, w1b_s, b1_s, zr_s, b2_s, nb2_s,
               eyr_s, nber_s, w_out, m_out, mode="full"):
    emba_s, embb_s = embt_halves
    if mode == "empty":
        return
    if True:
        if True:
            # BT = W1b @ embT  (f32, exact): psum half per matmul; each half
            # depends only on its own emb DMA, and the PSUM->SBUF copies run
            # on different engines so they overlap
            bt_ps = pp.tile([H, N], f32, tag="btp")
            nc.tensor.matmul(
                bt_ps[:, 0:512], w1b_s[:], emba_s[:], start=True, stop=True
            )
            nc.tensor.matmul(
                bt_ps[:, 512:1024], w1b_s[:], embb_s[:], start=True, stop=True
            )
            bt_s = cp.tile([H, N], f32, tag="bt")
            nc.vector.tensor_copy(bt_s[:, 0:512], bt_ps[:, 0:512])
            nc.scalar.copy(bt_s[:, 512:1024], bt_ps[:, 512:1024])

            # CT = W1a @ embT_rows + b1  (f32, exact)
            ct_ps = pp.tile([H, ROWS], f32, tag="ctp")
            nc.tensor.matmul(ct_ps[:], w1a_s[:], embr_s[:], start=True, stop=True)
            ct_s = cp.tile([H, ROWS], f32, tag="ct")
            nc.scalar.activation(
                ct_s[:], ct_ps[:], mybir.ActivationFunctionType.Identity, bias=b1_s[:]
            )

            # main loop: accumulate scores into PSUM [128 rows, 1024 cols]
            sc_ps = pp.tile([ROWS, N], f32, tag="scores")
            # initialize each scores bank with -BIG at the diagonal entries
            # (zeros elsewhere): out[k, j] = -BIG*eye[k, j]; keeps the
            # epilogue off the critical tail
            for h0 in (0, 512):
                nc.tensor.matmul(
                    sc_ps[:, h0 : h0 + 512],
                    nber_s[:],
                    eyr_s[:, h0 : h0 + 512],
                    start=True,
                    stop=False,
                )
            xfix = None
            if mode in ("nogen", "nogen_fixw", "nogen_w32", "nogen_1bank",
                        "nogen_fixw32", "nogen_b8", "nogen_256"):
                xfix = cp.tile([H, N], f32r, tag="xfix")
                nc.vector.tensor_copy(xfix[:, 0:256], zr_s[:])
            if mode == "nogen_256":
                # same total moving cols as nogen, but 512 MMs x 256 cols:
                # separates per-MM fixed overhead from cycle-rate
                lhsT = zr_s[:, 0:H]
                for i in range(2 * ROWS):
                    for c0 in (0, 256):
                        nc.tensor.matmul(
                            sc_ps[:, c0 : c0 + 256], lhsT, xfix[:, c0 : c0 + 256],
                            start=False, stop=(i == 2 * ROWS - 1 and c0 == 256),
                        )
                _epilogue(nc, cp, mybir, f32, u8, sc_ps, b2_s, nb2_s, w_out, m_out)
                return
            if mode == "nogen_bf16":
                # 256 MMs x 512 cols with bf16 moving + bf16 stationary:
                # tests whether the ~250ns/MM is f32r-specific or clock/overhead
                bf16 = mybir.dt.bfloat16
                xbf = cp.tile([H, N], bf16, tag="xbf")
                nc.vector.tensor_copy(xbf[:, 0:256], zr_s[:])
                zbf = cp.tile([H, H], bf16, tag="zbf")
                nc.vector.tensor_copy(zbf[:], zr_s[:, 0:H])
                for i in range(2 * ROWS):
                    nc.tensor.matmul(
                        sc_ps[:, 0:512], zbf[:], xbf[:, 0:512],
                        start=False, stop=(i == 2 * ROWS - 1),
                    )
                _epilogue(nc, cp, mybir, f32, u8, sc_ps, b2_s, nb2_s, w_out, m_out)
                return
            if mode == "nogen_1bank":
                # fixed 128-wide stationary, all MMs -> one PSUM bank
                lhsT = zr_s[:, 0:H]
                for i in range(2 * ROWS):
                    nc.tensor.matmul(
                        sc_ps[:, 0:512], lhsT, xfix[:, 0:512],
                        start=False, stop=(i == 2 * ROWS - 1),
                    )
                _epilogue(nc, cp, mybir, f32, u8, sc_ps, b2_s, nb2_s, w_out, m_out)
                return
            if mode == "nogen_fixw32":
                # fixed 32-wide stationary, all MMs -> one PSUM region
                lhsT = zr_s[:, 128:160]
                for i in range(2 * ROWS):
                    nc.tensor.matmul(
                        sc_ps[0:32, 0:512], lhsT, xfix[:, 0:512],
                        start=False, stop=(i == 2 * ROWS - 1),
                    )
                _epilogue(nc, cp, mybir, f32, u8, sc_ps, b2_s, nb2_s, w_out, m_out)
                return
            if mode == "nogen_b8":
                # sliding stationaries, banks switched every 8 rows
                for i0 in range(0, ROWS, 8):
                    for h0 in (0, 512):
                        for i in range(i0, i0 + 8):
                            lhsT = zr_s[:, H - i : 2 * H - i]
                            nc.tensor.matmul(
                                sc_ps[:, h0 : h0 + 512], lhsT, xfix[:, h0 : h0 + 512],
                                start=False,
                                stop=(i == ROWS - 1),
                            )
                _epilogue(nc, cp, mybir, f32, u8, sc_ps, b2_s, nb2_s, w_out, m_out)
                return
            if mode == "nogen_fixw":
                # PE-only, FIXED stationary: isolates LDWEIGHTS cost vs nogen
                lhsT = zr_s[:, 0:H]
                for i in range(ROWS):
                    nc.tensor.matmul(
                        sc_ps[:, 0:512], lhsT, xfix[:, 0:512],
                        start=False, stop=(i == ROWS - 1),
                    )
                    nc.tensor.matmul(
                        sc_ps[:, 512:1024], lhsT, xfix[:, 512:1024],
                        start=False, stop=(i == ROWS - 1),
                    )
                _epilogue(nc, cp, mybir, f32, u8, sc_ps, b2_s, nb2_s, w_out, m_out)
                return
            if mode == "nogen_w32":
                # PE-only, 32-wide sliding stationaries + tile_position groups
                for g in range(4):
                    for k in range(32):
                        lhsT = zr_s[:, H - k : H + 32 - k]
                        for h0 in (0, 512):
                            nc.tensor.matmul(
                                sc_ps[32 * g : 32 * g + 32, h0 : h0 + 512],
                                lhsT,
                                xfix[:, h0 : h0 + 512],
                                start=False,
                                stop=(k == 31),
                                tile_position=(0, 32 * g),
                            )
                _epilogue(nc, cp, mybir, f32, u8, sc_ps, b2_s, nb2_s, w_out, m_out)
                return
            if mode == "full2":
                # col-group tiled reduction: 32-wide stationaries, 4 strips
                for k in range(32):
                    for g in range(4):
                        i = 32 * g + k
                        x = xp.tile([H, N], f32r, tag="x")
                        if (i * 5) % 13 < 5:
                            nc.scalar.activation(
                                x[:],
                                bt_s[:],
                                mybir.ActivationFunctionType.Relu,
                                bias=ct_s[:, i : i + 1],
                            )
                        else:
                            nc.vector.tensor_scalar(
                                out=x[:],
                                in0=bt_s[:],
                                scalar1=ct_s[:, i : i + 1],
                                scalar2=0.0,
                                op0=mybir.AluOpType.add,
                                op1=mybir.AluOpType.max,
                            )
                        lhsT = zr_s[:, H - k : H + 32 - k]
                        for h0 in (0, 512):
                            nc.tensor.matmul(
                                sc_ps[32 * g : 32 * g + 32, h0 : h0 + 512],
                                lhsT,
                                x[:, h0 : h0 + 512],
                                start=(k == 0),
                                stop=(k == 31),
                                tile_position=(0, 32 * g),
                            )
                _epilogue(nc, cp, mybir, f32, u8, sc_ps, b2_s, nb2_s, w_out, m_out)
                return

            for i in range(ROWS):
                if mode != "nogen":
                    x = xp.tile([H, N], f32r, tag="x")
                    if mode == "actgen" or (mode != "dvegen" and i % 3 == 1):
                        # ACT path: relu(in + bias), ~1147ns
                        nc.scalar.activation(
                            x[:],
                            bt_s[:],
                            mybir.ActivationFunctionType.Relu,
                            bias=ct_s[:, i : i + 1],
                        )
                    else:
                        # DVE path: (in + c_i) then max(.,0), ~720ns
                        nc.vector.tensor_scalar(
                            out=x[:],
                            in0=bt_s[:],
                            scalar1=ct_s[:, i : i + 1],
                            scalar2=0.0,
                            op0=mybir.AluOpType.add,
                            op1=mybir.AluOpType.max,
                        )
                else:
                    x = xfix
                if mode == "nomm":
                    continue
                lhsT = zr_s[:, H - i : 2 * H - i]
                nc.tensor.matmul(
                    sc_ps[:, 0:512],
                    lhsT,
                    x[:, 0:512],
                    start=False,
                    stop=(i == ROWS - 1),
                )
                nc.tensor.matmul(
                    sc_ps[:, 512:1024],
                    lhsT,
                    x[:, 512:1024],
                    start=False,
                    stop=(i == ROWS - 1),
                )
            if mode == "nomm":
                return

            _epilogue(nc, cp, mybir, f32, u8, sc_ps, b2_s, nb2_s, w_out, m_out)


def _epilogue(nc, cp, mybir, f32, u8, sc_ps, b2_s, nb2_s, w_out, m_out):
    # diagonal score entries hold -BIG: sigmoid -> 0 weight, is_gt -> 0 mask
    sig_s = cp.tile([ROWS, N], mybir.dt.bfloat16, tag="sig")
    nc.scalar.activation(
        sig_s[:], sc_ps[:], mybir.ActivationFunctionType.Sigmoid, bias=b2_s[:]
    )
    nc.sync.dma_start(out=w_out[:], in_=sig_s[:])

    m_s = cp.tile([ROWS, N], u8, tag="m")
    nc.vector.tensor_scalar(
        out=m_s[:],
        in0=sc_ps[:],
        scalar1=nb2_s[:],
        scalar2=None,
        op0=mybir.AluOpType.is_gt,
    )
    nc.sync.dma_start(out=m_out[:], in_=m_s[:])


def _build_in_maps(inputs):
    node_emb = np.asarray(inputs["node_emb"], dtype=np.float32)
    W1 = np.asarray(inputs["W1"], dtype=np.float32)
    b1 = np.asarray(inputs["b1"], dtype=np.float32)
    W2 = np.asarray(inputs["W2"], dtype=np.float32)
    b2 = np.asarray(inputs["b2"], dtype=np.float32)

    emb_t = np.ascontiguousarray(node_emb.T)  # [H, N]
    w1a_t = np.ascontiguousarray(W1[:, :H].T)  # [e, h]
    w1b_t = np.ascontiguousarray(W1[:, H:].T)
    b1_col = np.ascontiguousarray(b1.reshape(H, 1))
    zbuf = np.zeros((H, 2 * H), dtype=np.float32)
    zbuf[:, H] = W2[0]
    b2v = np.float32(b2.reshape(-1)[0])
    b2_col = np.full((H, 1), b2v, dtype=np.float32)
    negb2_col = -b2_col

    negbig_eye = np.zeros((H, H), dtype=np.float32)
    np.fill_diagonal(negbig_eye, np.float32(-1e30))

    in_maps = []
    for c in range(NCORES):
        r0 = c * ROWS
        in_maps.append(
            {
                "emb_t": emb_t,
                "emb_rows_t": np.ascontiguousarray(emb_t[:, r0 : r0 + ROWS]),
                "w1a_t": w1a_t,
                "w1b_t": w1b_t,
                "b1_col": b1_col,
                "zbuf": zbuf,
                "b2_col": b2_col,
                "negb2_col": negb2_col,
                "rowcol": (r0 + np.arange(ROWS, dtype=np.float32)).reshape(ROWS, 1),
                "negbig_eye": negbig_eye,
            }
        )
    return in_maps


def _make_runner(nc):
    """Build a reusable jitted runner (mirrors bass2jax.run_bass_via_pjrt,
    but cached so repeated kernel() calls skip re-tracing/compiling)."""
    import jax
    import concourse.mybir as mybir
    from jax.sharding import Mesh, PartitionSpec

    try:
        from jax.experimental.shard_map import shard_map
    except ImportError:
        from jax.shard_map import shard_map

    from concourse.bass2jax import (
        _bass_exec_p,
        install_neuronx_cc_hook,
        partition_id_tensor,
    )

    install_neuronx_cc_hook()
    partition_name = nc.partition_id_tensor.name if nc.partition_id_tensor else None

    in_names, out_names, out_avals, zero_outs = [], [], [], []
    for alloc in nc.m.functions[0].allocations:
        if not isinstance(alloc, mybir.MemoryLocationSet):
            continue
        name = alloc.memorylocations[0].name
        if alloc.kind == "ExternalInput":
            if name != partition_name:
                in_names.append(name)
        elif alloc.kind == "ExternalOutput":
            out_names.append(name)
            shape = tuple(alloc.tensor_shape)
            dtype = mybir.dt.np(alloc.dtype)
            out_avals.append(jax.core.ShapedArray(shape, dtype))
            zero_outs.append(np.zeros(shape, dtype))
    n_params = len(in_names)
    all_in_names = list(in_names) + list(out_names)
    if partition_name is not None:
        all_in_names.append(partition_name)

    def _body(*args):
        operands = list(args)
        if partition_name is not None:
            operands.append(partition_id_tensor())
        return tuple(
            _bass_exec_p.bind(
                *operands,
                out_avals=tuple(out_avals),
                in_names=tuple(all_in_names),
                out_names=tuple(out_names),
                lowering_input_output_aliases=(),
                sim_require_finite=True,
                sim_require_nnan=True,
                nc=nc,
            )
        )

    devices = jax.devices()[:NCORES]
    mesh = Mesh(np.asarray(devices), ("core",))
    n_outs = len(out_avals)
    # only these inputs differ per core; the rest are replicated and ship
    # to the devices once instead of 8 concatenated copies
    per_core_names = {"emb_rows_t", "rowcol"}
    in_specs = tuple(
        PartitionSpec("core") if n in per_core_names else PartitionSpec(None)
        for n in in_names
    ) + (PartitionSpec("core"),) * n_outs
    out_specs = (PartitionSpec("core"),) * n_outs
    fn = jax.jit(
        shard_map(_body, mesh=mesh, in_specs=in_specs, out_specs=out_specs,
                  check_rep=False),
        keep_unused=True,
    )
    concat_zeros = [
        np.zeros((NCORES * z.shape[0], *z.shape[1:]), z.dtype) for z in zero_outs
    ]
    return fn, in_names, out_names, out_avals, concat_zeros, per_core_names


def _run_cached(in_maps):
    import jax

    if "runner" not in _cache:
        _cache["runner"] = _make_runner(_cache["nc"])
    fn, in_names, out_names, out_avals, concat_zeros, per_core_names = _cache["runner"]
    concat_in = [
        np.concatenate([np.asarray(m[name]) for m in in_maps], axis=0)
        if name in per_core_names
        else np.asarray(in_maps[0][name])
        for name in in_names
    ]
    out_arrs = fn(*concat_in, *concat_zeros)
    jax.block_until_ready(out_arrs)
    res = {}
    for i, name in enumerate(out_names):
        res[name] = np.asarray(out_arrs[i]).reshape(
            NCORES, *out_avals[i].shape
        )
    return res


def kernel(node_emb, W1, b1, W2, b2, temperature=None, **_ignored):
    import time

    if "nc" not in _cache:
        _cache["nc"] = _build()

    in_maps = _build_in_maps(
        {"node_emb": node_emb, "W1": W1, "b1": b1, "W2": W2, "b2": b2}
    )
    # the device occasionally reports NRT_EXEC_UNIT_UNRECOVERABLE if a prior
    # process wedged it; it self-recovers after ~30s, so retry those (and only
    # those) with backoff
    for attempt in range(3):
        try:
            res = _run_cached(in_maps)
            break
        except Exception as e:  # noqa: BLE001
            msg = str(e)
            transient = (
                "UNRECOVERABLE" in msg
                or "unrecoverable" in msg
                or "UNAVAILABLE" in msg
            )
            if attempt == 2 or not transient:
                raise
            time.sleep(30 * (attempt + 1))
    weights = np.concatenate(
        [np.asarray(res["w_out"][c]).astype(np.float32) for c in range(NCORES)],
        axis=0,
    )
    mask = np.concatenate([res["m_out"][c] for c in range(NCORES)], axis=0).astype(bool)
    return weights, mask

